# revision 54
# baseline (speedup 1.0000x reference)
"""Swin BasicLayer (depth=2 windowed attention) Trainium2 kernel.

Sharding: data-parallel over batch B=8 across 8 NeuronCores; weights
replicated. Each core runs both depths (regular + shifted windows) over
its [12544, 384] image in [C, token] layout.

Depths are interleaved band-by-band; the inter-depth intermediate lives
in SBUF as a 3-band bf16 ring (no HBM round trip, no barrier). Window
bias+mask are applied post-softmax-exp as a precomputed exp() factor.
"""
import numpy as np
import ml_dtypes

import concourse.bass as bass
import concourse.tile as tile
from concourse import bacc, mybir
from concourse.bass_utils import run_bass_kernel_spmd

f32 = mybir.dt.float32
f32r = mybir.dt.float32r
bf16 = mybir.dt.bfloat16
AF = mybir.ActivationFunctionType
ALU = mybir.AluOpType

B, H, W, C = 8, 112, 112, 384
NH, HD, WS = 12, 32, 7
N = WS * WS          # 49
L = H * W            # 12544
NBAND = H // WS      # 16
BAND = WS * W        # 784
HALF = BAND // 2     # 392
NWB = W // WS        # 16 windows per band
NG = NWB // 2        # 8 window-pair groups
DEPTH = 2
SHIFT = WS // 2      # 3


def _round_f32r(x):
    v = np.ascontiguousarray(x, np.float32).view(np.uint32)
    v = ((v.astype(np.uint64) + 0x800) & 0xFFFFF000).astype(np.uint32)
    return v.view(np.float32)


def _rel_pos_index():
    coords = np.stack(np.meshgrid(np.arange(WS), np.arange(WS), indexing='ij')).reshape(2, -1)
    rel = (coords[:, :, None] - coords[:, None, :]).transpose(1, 2, 0).copy()
    rel[..., 0] += WS - 1
    rel[..., 1] += WS - 1
    rel[..., 0] *= 2 * WS - 1
    return rel.sum(-1)


def _masks():
    """The 4 distinct [N, N] shifted-window masks by type 2*(i==15)+(j==15)."""
    ws, shift = WS, WS // 2
    img = np.zeros((H, W), dtype=np.float32)
    slices = (slice(0, -ws), slice(-ws, -shift), slice(-shift, None))
    cnt = 0
    for hs in slices:
        for wsl in slices:
            img[hs, wsl] = cnt
            cnt += 1
    mw = img.reshape(H // ws, ws, W // ws, ws).transpose(0, 2, 1, 3).reshape(-1, ws * ws)
    diff = mw[:, None, :] - mw[:, :, None]
    mask = np.where(diff != 0, -100.0, 0.0).astype(np.float32)  # [nW, N, N]
    nwr = H // ws
    m = {}
    for ti, widx in ((0, 0), (1, nwr - 1), (2, (nwr - 1) * nwr), (3, nwr * nwr - 1)):
        m[ti] = mask[widx]
    return m


def _band_ranges(i, shift):
    r0 = (WS * i + shift) % H
    n0 = min(WS, H - r0)
    rows = [(0, r0, n0)]
    if n0 < WS:
        rows.append((n0, 0, WS - n0))
    if shift == 0:
        cols = [(0, 0, W)]
    else:
        cols = [(0, shift, W - shift), (W - shift, 0, shift)]
    return rows, cols


def _build(nbands=NBAND, ndepth=DEPTH, simsafe=False, attn_dt=bf16):
    nc = bacc.Bacc("TRN2", target_bir_lowering=False, debug=False, num_devices=8)

    xin = nc.dram_tensor("xin", [C, L], f32r, kind="ExternalInput")
    xout = nc.dram_tensor("xout", [C, L], f32, kind="ExternalOutput")
    dwqk0 = nc.dram_tensor("wqk0", [C, 2 * C], f32r, kind="ExternalInput")
    dwqk1 = nc.dram_tensor("wqk1", [C, 2 * C], bf16, kind="ExternalInput")
    dwv0 = nc.dram_tensor("wv0", [C, C], f32r, kind="ExternalInput")
    dwv1 = nc.dram_tensor("wv1", [C, C], bf16, kind="ExternalInput")
    dwp = nc.dram_tensor("wp", [DEPTH, C, C], bf16, kind="ExternalInput")
    dbqk = nc.dram_tensor("bqk", [DEPTH, 2 * C], f32, kind="ExternalInput")
    dbp = nc.dram_tensor("bp", [DEPTH, C], f32, kind="ExternalInput")
    de0 = nc.dram_tensor("e0", [128, 2 * 12 * N], bf16, kind="ExternalInput")
    de1 = nc.dram_tensor("e1", [128, 8 * 12 * N], bf16, kind="ExternalInput")
    didn = nc.dram_tensor("idn", [128, 128], attn_dt, kind="ExternalInput")

    def evac_copy(use_dve, out_ap, in_ap):
        if use_dve:
            nc.vector.tensor_copy(out_ap, in_ap)
        else:
            nc.scalar.activation(out_ap, in_ap, AF.Identity, bias=0.0)

    def evac_bias(use_dve, out_ap, in_ap, bias_ap):
        if use_dve:
            nc.vector.tensor_scalar_add(out_ap, in_ap, bias_ap)
        else:
            nc.scalar.activation(out_ap, in_ap, AF.Identity, bias=bias_ap)

    def any_copy(eng, out_ap, in_ap):
        if eng == 0:
            nc.vector.tensor_copy(out_ap, in_ap)
        elif eng == 1:
            nc.scalar.activation(out_ap, in_ap, AF.Identity, bias=0.0)
        else:
            nc.gpsimd.tensor_copy(out_ap, in_ap)

    with tile.TileContext(nc) as tc:
        cpool = tc.alloc_tile_pool(name="const", bufs=1)
        p_xr = tc.alloc_tile_pool(name="xr", bufs=6)
        p_xw0 = tc.alloc_tile_pool(name="xw0", bufs=3)
        p_xw1 = tc.alloc_tile_pool(name="xw1", bufs=3)
        p_qkb = tc.alloc_tile_pool(name="qkb", bufs=18)
        p_outT = tc.alloc_tile_pool(name="outT", bufs=5)
        p_yr = tc.alloc_tile_pool(name="yr", bufs=3)
        p_pt = tc.alloc_tile_pool(name="pt", bufs=4)
        p_vt = tc.alloc_tile_pool(name="vt", bufs=18)
        p_on = tc.alloc_tile_pool(name="onat", bufs=4)
        p_rt = tc.alloc_tile_pool(name="rt", bufs=3)
        p_xm = tc.alloc_tile_pool(name="xm", bufs=9)
        p_stash = tc.alloc_tile_pool(name="stash", bufs=3)
        ps_proj = tc.alloc_tile_pool(name="psproj", bufs=2, space="PSUM")
        ps_v = tc.alloc_tile_pool(name="psv", bufs=2, space="PSUM")
        ps_s = tc.alloc_tile_pool(name="pss", bufs=1, space="PSUM")

        # constants
        wqk = [[cpool.tile([128, 2 * C], f32r if d == 0 else bf16,
                           tag=f"wqk{d}{cc}", name=f"wqk{d}{cc}") for cc in range(3)]
               for d in range(DEPTH)]
        wv = [[cpool.tile([128, C], f32r if d == 0 else bf16,
                          tag=f"wv{d}{cc}", name=f"wv{d}{cc}") for cc in range(3)]
              for d in range(DEPTH)]
        wp = [[cpool.tile([128, C], bf16, tag=f"wp{d}{cc}", name=f"wp{d}{cc}") for cc in range(3)]
              for d in range(DEPTH)]
        bqk = [cpool.tile([128, 6], f32, tag=f"bqk{d}", name=f"bqk{d}") for d in range(DEPTH)]
        bp = [cpool.tile([128, 3], f32, tag=f"bp{d}", name=f"bp{d}") for d in range(DEPTH)]
        e0 = cpool.tile([128, 2 * 12 * N], bf16, tag="e0", name="e0")
        e1 = cpool.tile([128, 8 * 12 * N], bf16, tag="e1", name="e1")
        idn = cpool.tile([128, 128], attn_dt, tag="idn", name="idn")
        for cc in range(3):
            nc.sync.dma_start(wqk[0][cc][:], dwqk0[cc * 128:(cc + 1) * 128, :])
            nc.sync.dma_start(wqk[1][cc][:], dwqk1[cc * 128:(cc + 1) * 128, :])
            nc.sync.dma_start(wv[0][cc][:], dwv0[cc * 128:(cc + 1) * 128, :])
            nc.sync.dma_start(wv[1][cc][:], dwv1[cc * 128:(cc + 1) * 128, :])
        for d in range(DEPTH):
            for cc in range(3):
                nc.sync.dma_start(wp[d][cc][:], dwp[d, cc * 128:(cc + 1) * 128, :])
            nc.sync.dma_start(bqk[d][:], dbqk[d].rearrange("(o p) -> p o", p=128))
            nc.sync.dma_start(bp[d][:], dbp[d].rearrange("(o p) -> p o", p=128))
        nc.sync.dma_start(e0[:], de0[:])
        nc.sync.dma_start(e1[:], de1[:])
        nc.sync.dma_start(idn[:], didn[:])

        xm_tiles = {}
        stash = [p_stash.tile([128, SHIFT * W], bf16, tag="stash", name=f"stash{cc}")
                 for cc in range(3)]

        def build_xw_d1(bi):
            """Gather shifted window-major band for depth 1 from the xm ring."""
            xw = [p_xw1.tile([128, BAND], bf16, tag="xw1", name="xw1") for _ in range(3)]
            lo = xm_tiles[bi]
            hi = xm_tiles[bi + 1] if bi + 1 < NBAND else stash
            for cc in range(3):
                lo_ = lo[cc][:]
                hi_ = hi[cc][:]
                xw_ = xw[cc][:]
                # pieces from lo rows 3..6 -> dst r 0..3; from hi rows 0..2 -> dst r 4..6
                pieces = [
                    (lo_, 3 * W + SHIFT, [[WS, NWB], [W, 4], [1, 4]],
                     0, [[N, NWB], [WS, 4], [1, 4]]),
                    (lo_, 3 * W + WS, [[WS, NWB - 1], [W, 4], [1, 3]],
                     4, [[N, NWB - 1], [WS, 4], [1, 3]]),
                    (lo_, 3 * W, [[W, 4], [1, 3]],
                     N * (NWB - 1) + 4, [[WS, 4], [1, 3]]),
                    (hi_, SHIFT, [[WS, NWB], [W, 3], [1, 4]],
                     4 * WS, [[N, NWB], [WS, 3], [1, 4]]),
                    (hi_, WS, [[WS, NWB - 1], [W, 3], [1, 3]],
                     4 * WS + 4, [[N, NWB - 1], [WS, 3], [1, 3]]),
                    (hi_, 0, [[W, 3], [1, 3]],
                     N * (NWB - 1) + 4 * WS + 4, [[WS, 3], [1, 3]]),
                ]
                engs = [cc % 2, (cc + 1) % 2, 2, (cc + 1) % 2, cc % 2, 2]
                for pi, (sap, soff, sdims, doff, ddims) in enumerate(pieces):
                    in_ap = bass.AP(sap.tensor, sap.offset + soff, [sap.ap[0]] + sdims)
                    out_ap = bass.AP(xw_.tensor, xw_.offset + doff, [xw_.ap[0]] + ddims)
                    any_copy(engs[pi], out_ap, in_ap)
            return xw

        def process_band(d, bi):
            if d == 0:
                rows, cols = _band_ranges(bi, 0)
                xr = [p_xr.tile([128, BAND], f32r, tag="xr", name="xr") for _ in range(3)]
                xw = [p_xw0.tile([128, BAND], f32r, tag="xw0", name="xw0") for _ in range(3)]
                for cc in range(3):
                    xr3 = xr[cc][:].rearrange("p (r c) -> p r c", r=WS)
                    src3 = xin[cc * 128:(cc + 1) * 128, :].rearrange(
                        "p (r c) -> p r c", r=H)
                    for (dr, sr, nr) in rows:
                        for (dc, sc, ncl) in cols:
                            nc.sync.dma_start(xr3[:, dr:dr + nr, dc:dc + ncl],
                                              src3[:, sr:sr + nr, sc:sc + ncl])
                    # raster (r, 7w+j) -> window-major (49w + 7r + j), in halves
                    in_ap = xr[cc][:].rearrange("p (r w j) -> p w r j", r=WS, w=NWB, j=WS)
                    out_ap = xw[cc][:].rearrange("p (w r j) -> p w r j", w=NWB, r=WS, j=WS)
                    evac_copy((bi + cc) % 2, out_ap[:, 0:NG, :, :], in_ap[:, 0:NG, :, :])
                    evac_copy((bi + cc + 1) % 2, out_ap[:, NG:NWB, :, :], in_ap[:, NG:NWB, :, :])
            else:
                xw = build_xw_d1(bi)
            # ---- q,k projection
            qkb = [p_qkb.tile([128, BAND], attn_dt, tag="qkb", name="qkb") for _ in range(6)]
            for half in range(2):
                for oc in range(6):
                    ps = ps_proj.tile([128, HALF], f32, tag="psproj", name="psproj", padded_shape=[128, 512])
                    for cc in range(3):
                        nc.tensor.matmul(
                            ps[:], wqk[d][cc][:, oc * 128:(oc + 1) * 128],
                            xw[cc][:, half * HALF:(half + 1) * HALF],
                            start=(cc == 0), stop=(cc == 2))
                    evac_bias((oc + half) % 2,
                              qkb[oc][:, half * HALF:(half + 1) * HALF],
                              ps[:], bqk[d][:, oc:oc + 1])
            # ---- window-pair groups
            outT = p_outT.tile([128, 3 * BAND], bf16, tag="outT", name="outT")
            # ---- V projection for all 8 groups up front
            vts = []
            for g in range(NG):
                vt = p_vt.tile([128, 2 * 12 * 33], attn_dt, tag="vt", name="vt")
                vp = ps_v.tile([128, C], f32, tag="psv", name="psv", padded_shape=[128, 512])
                for cc in range(3):
                    nc.tensor.matmul(vp[0:98, :],
                                     xw[cc][:, 98 * g:98 * g + 98],
                                     wv[d][cc][:],
                                     start=(cc == 0), stop=(cc == 2))
                iv = vp[0:98, :].rearrange("p (h e) -> p h e", h=12)
                ov = vt[0:98, 0:396].rearrange("p (h e) -> p h e", e=33)[:, :, 0:32]
                evac_copy(g % 2, ov, iv)
                nc.vector.memset(
                    vt[:, 0:396].rearrange("p (h e) -> p h e", e=33)[:, :, 32:33],
                    1.0)
                nc.gpsimd.dma_start(vt[64:113, 396:792], vt[49:98, 0:396])
                vts.append(vt)
            for gp in range(NG // 2):
                # e-tile pair block: d0 -> e0 (dup); d1 -> 4 variants
                if d == 0:
                    et, blk = e0, 0
                elif gp < NG // 2 - 1:
                    et, blk = e1, (0 if bi < NBAND - 1 else 2)
                else:
                    et, blk = e1, (1 if bi < NBAND - 1 else 3)
                ebase = blk * 2 * 12 * N
                sp = ps_s.tile([128, 2048], f32, tag="pss", name="pss")
                for gg in range(2):
                    g = 2 * gp + gg
                    for s in range(2):
                        w = 2 * g + s
                        for h in range(NH):
                            po = 32 * (h % 4)
                            kT = qkb[3 + h // 4][po:po + 32, N * w:N * w + N]
                            qT = qkb[h // 4][po:po + 32, N * w:N * w + N]
                            out = sp[64 * s:64 * s + 49,
                                     512 * (h % 4) + N * (h // 4 + 3 * gg):
                                     512 * (h % 4) + N * (h // 4 + 3 * gg) + N]
                            nc.tensor.matmul(out, kT, qT, start=True, stop=True,
                                             tile_position=(po, 64 * s))
                # single exp for the pair: psum -> bf16 pt (52-pitch x2)
                pt = p_pt.tile([128, 2 * 12 * 52], attn_dt, tag="pt", name="pt")
                if simsafe:
                    nc.vector.memset(pt[32:64, :], 0.0)
                    nc.vector.memset(pt[96:128, :], 0.0)
                exp_in = bass.AP(sp[:].tensor, sp[:].offset,
                                 [sp[:].ap[0], [512, 4], [N, 6], [1, N]])
                po_ = pt[:].rearrange("p f -> p f")
                exp_out = bass.AP(po_.tensor, po_.offset,
                                  [po_.ap[0], [52, 4], [4 * 52, 6], [1, N]])
                nc.scalar.activation(exp_out, exp_in, AF.Exp)
                # EB multiply (bias+mask in prob domain), alternating engine
                ptm = p_pt.tile([128, 2 * 12 * 52], attn_dt, tag="ptm", name="ptm")
                ea = et[:].rearrange("p f -> p f")
                eb_in = bass.AP(ea.tensor, ea.offset + ebase,
                                [ea.ap[0], [N, 4], [4 * N, 6], [1, N]])
                pt_in = bass.AP(po_.tensor, po_.offset,
                                [po_.ap[0], [52, 4], [4 * 52, 6], [1, N]])
                pm_ = ptm[:].rearrange("p f -> p f")
                for gg_ in range(2):
                    eb_in_g = bass.AP(ea.tensor, ea.offset + ebase + 12 * N * gg_,
                                      [ea.ap[0], [N, 4], [4 * N, 3], [1, N]])
                    pt_in_g = bass.AP(po_.tensor, po_.offset + 12 * 52 * gg_,
                                      [po_.ap[0], [52, 4], [4 * 52, 3], [1, N]])
                    ptm_out_g = bass.AP(pm_.tensor, pm_.offset + 12 * 52 * gg_,
                                        [pm_.ap[0], [52, 4], [4 * 52, 3], [1, N]])
                    nc.vector.tensor_tensor(ptm_out_g, pt_in_g, eb_in_g, ALU.mult)
                for gg in range(2):
                    g = 2 * gp + gg
                    vt = vts[g]
                    pbase = gg * 12 * 52
                    # AV (+ fused row-sums via ones column of vt)
                    av = ps_proj.tile([128, 12 * 33], f32, tag="psproj", name="psav", padded_shape=[128, 512])
                    if simsafe:
                        nc.vector.memset(av[32:64, :], 0.0)
                        nc.vector.memset(av[96:128, :], 0.0)
                    for s in range(2):
                        for h in range(NH):
                            nc.tensor.matmul(
                                av[64 * s:64 * s + 49, 33 * h:33 * h + 33],
                                ptm[64 * s:64 * s + 49,
                                    pbase + 52 * h:pbase + 52 * h + N],
                                vt[64 * s:64 * s + 49,
                                   396 * s + 33 * h:396 * s + 33 * h + 33],
                                start=True, stop=True,
                                tile_position=(64 * s, 64 * s))
                    # normalize
                    rt = p_rt.tile([128, 12], f32, tag="rt", name="rt")
                    nc.vector.reciprocal(
                        rt[:].rearrange("p (h e) -> p h e", e=1),
                        av[:].rearrange("p (h e) -> p h e", e=33)[:, :, 32:33])
                    on = p_on.tile([128, C], attn_dt, tag="onat", name="onat")
                    rap = rt[:]
                    rbc = bass.AP(rap.tensor, rap.offset, [rap.ap[0], [1, 12], [0, 32]])
                    nc.vector.tensor_tensor(
                        on[:].rearrange("p (h e) -> p h e", e=32),
                        av[:].rearrange("p (h e) -> p h e", e=33)[:, :, 0:32],
                        rbc, ALU.mult)
                    # transpose out [n, c] -> [c, n]: full-width, both windows at once
                    tp = ps_v.tile([128, 3 * 128], attn_dt, tag="psv", name="psv",
                                   padded_shape=[128, 512])
                    for cc in range(3):
                        nc.tensor.transpose(
                            tp[:, 128 * cc:128 * cc + 128],
                            on[:, cc * 128:(cc + 1) * 128],
                            idn[:])
                    oT = outT[:].rearrange("p (c t) -> p c t", c=3)
                    for s in range(2):
                        out_ap = bass.AP(oT.tensor, oT.offset + 98 * g + N * s,
                                         [oT.ap[0], [BAND, 3], [1, N]])
                        in_ap = bass.AP(tp[:].tensor, tp[:].offset + 64 * s,
                                        [tp[:].ap[0], [128, 3], [1, N]])
                        evac_copy(s % 2, out_ap, in_ap)
            # ---- output projection (+ window-major -> raster relayout)
            if d == 0:
                xm = [p_xm.tile([128, BAND], bf16, tag="xm", name="xm") for _ in range(3)]
                xm_tiles[bi] = xm
                ydst_t = xm
            else:
                yr = [p_yr.tile([128, BAND], f32, tag="yr", name="yr") for _ in range(3)]
                ydst_t = yr
            for half in range(2):
                for oc in range(3):
                    ps = ps_proj.tile([128, HALF], f32, tag="psproj", name="psproj", padded_shape=[128, 512])
                    for cc in range(3):
                        nc.tensor.matmul(
                            ps[:], wp[d][cc][:, oc * 128:(oc + 1) * 128],
                            outT[:, cc * BAND + half * HALF:
                                 cc * BAND + (half + 1) * HALF],
                            start=(cc == 0), stop=(cc == 2))
                    in_ap = ps[:].rearrange("p (w r j) -> p w r j",
                                            w=NG, r=WS, j=WS)
                    out_ap = ydst_t[oc][:].rearrange(
                        "p (r w j) -> p w r j", r=WS, w=NWB,
                        j=WS)[:, NG * half:NG * half + NG, :, :]
                    evac_bias((oc + half) % 2, out_ap, in_ap,
                              bp[d][:, oc:oc + 1])
            if d == 0:
                if bi == 0:
                    for cc in range(3):
                        nc.gpsimd.tensor_copy(stash[cc][:], xm_tiles[0][cc][:, 0:SHIFT * W])
            else:
                rows, cols = _band_ranges(bi, SHIFT)
                for oc in range(3):
                    yr3 = ydst_t[oc][:].rearrange("p (r c) -> p r c", r=WS)
                    dst3 = xout[oc * 128:(oc + 1) * 128, :].rearrange(
                        "p (r c) -> p r c", r=H)
                    for (dr, sr, nr) in rows:
                        for (dc, sc, ncl) in cols:
                            nc.sync.dma_start(dst3[:, sr:sr + nr, sc:sc + ncl],
                                              yr3[:, dr:dr + nr, dc:dc + ncl])

        for step in range(nbands + 2):
            if step < nbands:
                process_band(0, step)
            if step >= 2 and ndepth > 1:
                process_band(1, step - 2)

        for p in (ps_s, ps_v, ps_proj, p_stash, p_xm, p_rt, p_on, p_vt,
                  p_pt, p_yr, p_outT, p_qkb, p_xw1, p_xw0, p_xr, cpool):
            p.release()

    nc.compile()
    return nc


_NC = None


def _get_nc():
    global _NC
    if _NC is None:
        _NC = _build()
    return _NC


def _host_prep(qkv_w, qkv_b, proj_w, proj_b, rpb_table):
    scale = HD ** -0.5
    rpi = _rel_pos_index()
    masks = _masks()
    common = {}
    wqk = np.zeros((DEPTH, C, 2 * C), np.float32)
    wvv = np.zeros((DEPTH, C, C), np.float32)
    wpp = np.zeros((DEPTH, C, C), np.float32)
    bqk = np.zeros((DEPTH, 2 * C), np.float32)
    bpp = np.zeros((DEPTH, C), np.float32)
    for d in range(DEPTH):
        wq = qkv_w[d][:2 * C].T.copy()        # [C, 2C] (q then k)
        wq[:, :C] *= scale
        wqk[d] = wq
        wvv[d] = qkv_w[d][2 * C:].T
        wpp[d] = proj_w[d].T
        bq = qkv_b[d][:2 * C].copy()
        bq[:C] *= scale
        bqk[d] = bq
        bv = qkv_b[d][2 * C:]
        bpp[d] = proj_b[d] + proj_w[d] @ bv
    common["wqk0"] = _round_f32r(wqk[0])
    common["wqk1"] = wqk[1].astype(ml_dtypes.bfloat16)
    common["wv0"] = _round_f32r(wvv[0])
    common["wv1"] = wvv[1].astype(ml_dtypes.bfloat16)
    common["wp"] = wpp.astype(ml_dtypes.bfloat16)
    common["bqk"] = bqk
    common["bp"] = bpp

    # E tiles: rows 0-48 -> m, rows 64-112 -> m-64; value exp(bias[h,n,m]+mask[n,m])
    def etile(d, type_a, type_b):
        bias = rpb_table[d][rpi]              # [N, N, NH]
        t = np.zeros((128, 12 * N), np.float32)
        for s, ty in ((0, type_a), (1, type_b)):
            bm = bias + (masks[ty][:, :, None] if ty is not None else 0.0)
            ev = bm.transpose(2, 1, 0)   # [NH, m, n] (log domain)
            blk = ev.transpose(1, 0, 2).reshape(N, 12 * N)  # row m, col h*N+n
            t[64 * s:64 * s + N, :] = blk
        return t

    eb0 = np.exp(etile(0, None, None))
    common["e0"] = np.concatenate([eb0, eb0], axis=1).astype(ml_dtypes.bfloat16)
    # pair variants: [a|a], [a|c], [b|b], [b|d] where a,b,c,d = blocks
    # (0,0),(2,2),(0,1),(2,3) of the old layout
    ba = np.exp(etile(1, 0, 0))
    bb = np.exp(etile(1, 2, 2))
    bc = np.exp(etile(1, 0, 1))
    bd = np.exp(etile(1, 2, 3))
    e1 = np.concatenate([ba, ba, ba, bc, bb, bb, bb, bd], axis=1)
    common["e1"] = e1.astype(ml_dtypes.bfloat16)

    idn = np.zeros((128, 128), np.float32)
    for p in list(range(N)) + list(range(64, 64 + N)):
        idn[p, p] = 1.0
    common["idn"] = idn.astype(ml_dtypes.bfloat16)
    return common


def kernel(x, qkv_w, qkv_b, proj_w, proj_b, rpb_table, H=None, W=None):
    x = np.asarray(x, np.float32)
    qkv_w = np.asarray(qkv_w, np.float32)
    qkv_b = np.asarray(qkv_b, np.float32)
    proj_w = np.asarray(proj_w, np.float32)
    proj_b = np.asarray(proj_b, np.float32)
    rpb_table = np.asarray(rpb_table, np.float32)

    nc = _get_nc()
    common = _host_prep(qkv_w, qkv_b, proj_w, proj_b, rpb_table)
    in_maps = []
    for b in range(B):
        m = dict(common)
        m["xin"] = _round_f32r(np.ascontiguousarray(x[b].T))
        in_maps.append(m)
    res = run_bass_kernel_spmd(nc, in_maps, core_ids=list(range(B)))
    out = np.stack([np.ascontiguousarray(res.results[b]["xout"].T)
                    for b in range(B)])
    return out.astype(np.float32)


# revision 55
# speedup vs baseline: 1.0001x; 1.0001x over previous
"""Swin BasicLayer (depth=2 windowed attention) Trainium2 kernel.

Sharding: data-parallel over batch B=8 across 8 NeuronCores; weights
replicated. Each core runs both depths (regular + shifted windows) over
its [12544, 384] image in [C, token] layout.

Depths are interleaved band-by-band; the inter-depth intermediate lives
in SBUF as a 3-band bf16 ring (no HBM round trip, no barrier). Window
bias+mask are applied post-softmax-exp as a precomputed exp() factor.
"""
import numpy as np
import ml_dtypes

import concourse.bass as bass
import concourse.tile as tile
from concourse import bacc, mybir
from concourse.bass_utils import run_bass_kernel_spmd

f32 = mybir.dt.float32
f32r = mybir.dt.float32r
bf16 = mybir.dt.bfloat16
AF = mybir.ActivationFunctionType
ALU = mybir.AluOpType

B, H, W, C = 8, 112, 112, 384
NH, HD, WS = 12, 32, 7
N = WS * WS          # 49
L = H * W            # 12544
NBAND = H // WS      # 16
BAND = WS * W        # 784
HALF = BAND // 2     # 392
NWB = W // WS        # 16 windows per band
NG = NWB // 2        # 8 window-pair groups
DEPTH = 2
SHIFT = WS // 2      # 3


def _round_f32r(x):
    v = np.ascontiguousarray(x, np.float32).view(np.uint32)
    v = ((v.astype(np.uint64) + 0x800) & 0xFFFFF000).astype(np.uint32)
    return v.view(np.float32)


def _rel_pos_index():
    coords = np.stack(np.meshgrid(np.arange(WS), np.arange(WS), indexing='ij')).reshape(2, -1)
    rel = (coords[:, :, None] - coords[:, None, :]).transpose(1, 2, 0).copy()
    rel[..., 0] += WS - 1
    rel[..., 1] += WS - 1
    rel[..., 0] *= 2 * WS - 1
    return rel.sum(-1)


def _masks():
    """The 4 distinct [N, N] shifted-window masks by type 2*(i==15)+(j==15)."""
    ws, shift = WS, WS // 2
    img = np.zeros((H, W), dtype=np.float32)
    slices = (slice(0, -ws), slice(-ws, -shift), slice(-shift, None))
    cnt = 0
    for hs in slices:
        for wsl in slices:
            img[hs, wsl] = cnt
            cnt += 1
    mw = img.reshape(H // ws, ws, W // ws, ws).transpose(0, 2, 1, 3).reshape(-1, ws * ws)
    diff = mw[:, None, :] - mw[:, :, None]
    mask = np.where(diff != 0, -100.0, 0.0).astype(np.float32)  # [nW, N, N]
    nwr = H // ws
    m = {}
    for ti, widx in ((0, 0), (1, nwr - 1), (2, (nwr - 1) * nwr), (3, nwr * nwr - 1)):
        m[ti] = mask[widx]
    return m


def _band_ranges(i, shift):
    r0 = (WS * i + shift) % H
    n0 = min(WS, H - r0)
    rows = [(0, r0, n0)]
    if n0 < WS:
        rows.append((n0, 0, WS - n0))
    if shift == 0:
        cols = [(0, 0, W)]
    else:
        cols = [(0, shift, W - shift), (W - shift, 0, shift)]
    return rows, cols


def _build(nbands=NBAND, ndepth=DEPTH, simsafe=False, attn_dt=bf16):
    nc = bacc.Bacc("TRN2", target_bir_lowering=False, debug=False, num_devices=8)

    xin = nc.dram_tensor("xin", [C, L], f32r, kind="ExternalInput")
    xout = nc.dram_tensor("xout", [C, L], f32, kind="ExternalOutput")
    dwqk0 = nc.dram_tensor("wqk0", [C, 2 * C], f32r, kind="ExternalInput")
    dwqk1 = nc.dram_tensor("wqk1", [C, 2 * C], bf16, kind="ExternalInput")
    dwv0 = nc.dram_tensor("wv0", [C, C], f32r, kind="ExternalInput")
    dwv1 = nc.dram_tensor("wv1", [C, C], bf16, kind="ExternalInput")
    dwp = nc.dram_tensor("wp", [DEPTH, C, C], bf16, kind="ExternalInput")
    dbqk = nc.dram_tensor("bqk", [DEPTH, 2 * C], f32, kind="ExternalInput")
    dbp = nc.dram_tensor("bp", [DEPTH, C], f32, kind="ExternalInput")
    de0 = nc.dram_tensor("e0", [128, 2 * 12 * N], bf16, kind="ExternalInput")
    de1 = nc.dram_tensor("e1", [128, 8 * 12 * N], bf16, kind="ExternalInput")
    didn = nc.dram_tensor("idn", [128, 128], attn_dt, kind="ExternalInput")

    def evac_copy(use_dve, out_ap, in_ap):
        if use_dve:
            nc.vector.tensor_copy(out_ap, in_ap)
        else:
            nc.scalar.activation(out_ap, in_ap, AF.Identity, bias=0.0)

    def evac_bias(use_dve, out_ap, in_ap, bias_ap):
        if use_dve:
            nc.vector.tensor_scalar_add(out_ap, in_ap, bias_ap)
        else:
            nc.scalar.activation(out_ap, in_ap, AF.Identity, bias=bias_ap)

    def any_copy(eng, out_ap, in_ap):
        if eng == 0:
            nc.vector.tensor_copy(out_ap, in_ap)
        elif eng == 1:
            nc.scalar.activation(out_ap, in_ap, AF.Identity, bias=0.0)
        else:
            nc.gpsimd.tensor_copy(out_ap, in_ap)

    with tile.TileContext(nc) as tc:
        cpool = tc.alloc_tile_pool(name="const", bufs=1)
        p_xr = tc.alloc_tile_pool(name="xr", bufs=6)
        p_xw0 = tc.alloc_tile_pool(name="xw0", bufs=3)
        p_xw1 = tc.alloc_tile_pool(name="xw1", bufs=3)
        p_qkb = tc.alloc_tile_pool(name="qkb", bufs=18)
        p_outT = tc.alloc_tile_pool(name="outT", bufs=5)
        p_yr = tc.alloc_tile_pool(name="yr", bufs=4)
        p_pt = tc.alloc_tile_pool(name="pt", bufs=4)
        p_vt = tc.alloc_tile_pool(name="vt", bufs=18)
        p_on = tc.alloc_tile_pool(name="onat", bufs=4)
        p_rt = tc.alloc_tile_pool(name="rt", bufs=3)
        p_xm = tc.alloc_tile_pool(name="xm", bufs=12)
        p_stash = tc.alloc_tile_pool(name="stash", bufs=3)
        ps_proj = tc.alloc_tile_pool(name="psproj", bufs=2, space="PSUM")
        ps_v = tc.alloc_tile_pool(name="psv", bufs=2, space="PSUM")
        ps_s = tc.alloc_tile_pool(name="pss", bufs=1, space="PSUM")

        # constants
        wqk = [[cpool.tile([128, 2 * C], f32r if d == 0 else bf16,
                           tag=f"wqk{d}{cc}", name=f"wqk{d}{cc}") for cc in range(3)]
               for d in range(DEPTH)]
        wv = [[cpool.tile([128, C], f32r if d == 0 else bf16,
                          tag=f"wv{d}{cc}", name=f"wv{d}{cc}") for cc in range(3)]
              for d in range(DEPTH)]
        wp = [[cpool.tile([128, C], bf16, tag=f"wp{d}{cc}", name=f"wp{d}{cc}") for cc in range(3)]
              for d in range(DEPTH)]
        bqk = [cpool.tile([128, 6], f32, tag=f"bqk{d}", name=f"bqk{d}") for d in range(DEPTH)]
        bp = [cpool.tile([128, 3], f32, tag=f"bp{d}", name=f"bp{d}") for d in range(DEPTH)]
        e0 = cpool.tile([128, 2 * 12 * N], bf16, tag="e0", name="e0")
        e1 = cpool.tile([128, 8 * 12 * N], bf16, tag="e1", name="e1")
        idn = cpool.tile([128, 128], attn_dt, tag="idn", name="idn")
        for cc in range(3):
            nc.sync.dma_start(wqk[0][cc][:], dwqk0[cc * 128:(cc + 1) * 128, :])
            nc.sync.dma_start(wqk[1][cc][:], dwqk1[cc * 128:(cc + 1) * 128, :])
            nc.sync.dma_start(wv[0][cc][:], dwv0[cc * 128:(cc + 1) * 128, :])
            nc.sync.dma_start(wv[1][cc][:], dwv1[cc * 128:(cc + 1) * 128, :])
        for d in range(DEPTH):
            for cc in range(3):
                nc.sync.dma_start(wp[d][cc][:], dwp[d, cc * 128:(cc + 1) * 128, :])
            nc.sync.dma_start(bqk[d][:], dbqk[d].rearrange("(o p) -> p o", p=128))
            nc.sync.dma_start(bp[d][:], dbp[d].rearrange("(o p) -> p o", p=128))
        nc.sync.dma_start(e0[:], de0[:])
        nc.sync.dma_start(e1[:], de1[:])
        nc.sync.dma_start(idn[:], didn[:])

        xm_tiles = {}
        stash = [p_stash.tile([128, SHIFT * W], bf16, tag="stash", name=f"stash{cc}")
                 for cc in range(3)]

        def build_xw_d1(bi):
            """Gather shifted window-major band for depth 1 from the xm ring."""
            xw = [p_xw1.tile([128, BAND], bf16, tag="xw1", name="xw1") for _ in range(3)]
            lo = xm_tiles[bi]
            hi = xm_tiles[bi + 1] if bi + 1 < NBAND else stash
            for cc in range(3):
                lo_ = lo[cc][:]
                hi_ = hi[cc][:]
                xw_ = xw[cc][:]
                # pieces from lo rows 3..6 -> dst r 0..3; from hi rows 0..2 -> dst r 4..6
                pieces = [
                    (lo_, 3 * W + SHIFT, [[WS, NWB], [W, 4], [1, 4]],
                     0, [[N, NWB], [WS, 4], [1, 4]]),
                    (lo_, 3 * W + WS, [[WS, NWB - 1], [W, 4], [1, 3]],
                     4, [[N, NWB - 1], [WS, 4], [1, 3]]),
                    (lo_, 3 * W, [[W, 4], [1, 3]],
                     N * (NWB - 1) + 4, [[WS, 4], [1, 3]]),
                    (hi_, SHIFT, [[WS, NWB], [W, 3], [1, 4]],
                     4 * WS, [[N, NWB], [WS, 3], [1, 4]]),
                    (hi_, WS, [[WS, NWB - 1], [W, 3], [1, 3]],
                     4 * WS + 4, [[N, NWB - 1], [WS, 3], [1, 3]]),
                    (hi_, 0, [[W, 3], [1, 3]],
                     N * (NWB - 1) + 4 * WS + 4, [[WS, 3], [1, 3]]),
                ]
                engs = [cc % 2, (cc + 1) % 2, 2, (cc + 1) % 2, cc % 2, 2]
                for pi, (sap, soff, sdims, doff, ddims) in enumerate(pieces):
                    in_ap = bass.AP(sap.tensor, sap.offset + soff, [sap.ap[0]] + sdims)
                    out_ap = bass.AP(xw_.tensor, xw_.offset + doff, [xw_.ap[0]] + ddims)
                    any_copy(engs[pi], out_ap, in_ap)
            return xw

        def process_band(d, bi):
            if d == 0:
                rows, cols = _band_ranges(bi, 0)
                xr = [p_xr.tile([128, BAND], f32r, tag="xr", name="xr") for _ in range(3)]
                xw = [p_xw0.tile([128, BAND], f32r, tag="xw0", name="xw0") for _ in range(3)]
                for cc in range(3):
                    xr3 = xr[cc][:].rearrange("p (r c) -> p r c", r=WS)
                    src3 = xin[cc * 128:(cc + 1) * 128, :].rearrange(
                        "p (r c) -> p r c", r=H)
                    for (dr, sr, nr) in rows:
                        for (dc, sc, ncl) in cols:
                            nc.sync.dma_start(xr3[:, dr:dr + nr, dc:dc + ncl],
                                              src3[:, sr:sr + nr, sc:sc + ncl])
                    # raster (r, 7w+j) -> window-major (49w + 7r + j), in halves
                    in_ap = xr[cc][:].rearrange("p (r w j) -> p w r j", r=WS, w=NWB, j=WS)
                    out_ap = xw[cc][:].rearrange("p (w r j) -> p w r j", w=NWB, r=WS, j=WS)
                    evac_copy((bi + cc) % 2, out_ap[:, 0:NG, :, :], in_ap[:, 0:NG, :, :])
                    evac_copy((bi + cc + 1) % 2, out_ap[:, NG:NWB, :, :], in_ap[:, NG:NWB, :, :])
            else:
                xw = build_xw_d1(bi)
            # ---- q,k projection
            qkb = [p_qkb.tile([128, BAND], attn_dt, tag="qkb", name="qkb") for _ in range(6)]
            for half in range(2):
                for oc in range(6):
                    ps = ps_proj.tile([128, HALF], f32, tag="psproj", name="psproj", padded_shape=[128, 512])
                    for cc in range(3):
                        nc.tensor.matmul(
                            ps[:], wqk[d][cc][:, oc * 128:(oc + 1) * 128],
                            xw[cc][:, half * HALF:(half + 1) * HALF],
                            start=(cc == 0), stop=(cc == 2))
                    evac_bias((oc + half) % 2,
                              qkb[oc][:, half * HALF:(half + 1) * HALF],
                              ps[:], bqk[d][:, oc:oc + 1])
            # ---- window-pair groups
            outT = p_outT.tile([128, 3 * BAND], bf16, tag="outT", name="outT")
            # ---- V projection for all 8 groups up front
            vts = []
            for g in range(NG):
                vt = p_vt.tile([128, 2 * 12 * 33], attn_dt, tag="vt", name="vt")
                vp = ps_v.tile([128, C], f32, tag="psv", name="psv", padded_shape=[128, 512])
                for cc in range(3):
                    nc.tensor.matmul(vp[0:98, :],
                                     xw[cc][:, 98 * g:98 * g + 98],
                                     wv[d][cc][:],
                                     start=(cc == 0), stop=(cc == 2))
                iv = vp[0:98, :].rearrange("p (h e) -> p h e", h=12)
                ov = vt[0:98, 0:396].rearrange("p (h e) -> p h e", e=33)[:, :, 0:32]
                evac_copy(g % 2, ov, iv)
                nc.vector.memset(
                    vt[:, 0:396].rearrange("p (h e) -> p h e", e=33)[:, :, 32:33],
                    1.0)
                nc.gpsimd.dma_start(vt[64:113, 396:792], vt[49:98, 0:396])
                vts.append(vt)
            for gp in range(NG // 2):
                # e-tile pair block: d0 -> e0 (dup); d1 -> 4 variants
                if d == 0:
                    et, blk = e0, 0
                elif gp < NG // 2 - 1:
                    et, blk = e1, (0 if bi < NBAND - 1 else 2)
                else:
                    et, blk = e1, (1 if bi < NBAND - 1 else 3)
                ebase = blk * 2 * 12 * N
                sp = ps_s.tile([128, 2048], f32, tag="pss", name="pss")
                for gg in range(2):
                    g = 2 * gp + gg
                    for s in range(2):
                        w = 2 * g + s
                        for h in range(NH):
                            po = 32 * (h % 4)
                            kT = qkb[3 + h // 4][po:po + 32, N * w:N * w + N]
                            qT = qkb[h // 4][po:po + 32, N * w:N * w + N]
                            out = sp[64 * s:64 * s + 49,
                                     512 * (h % 4) + N * (h // 4 + 3 * gg):
                                     512 * (h % 4) + N * (h // 4 + 3 * gg) + N]
                            nc.tensor.matmul(out, kT, qT, start=True, stop=True,
                                             tile_position=(po, 64 * s))
                # single exp for the pair: psum -> bf16 pt (52-pitch x2)
                pt = p_pt.tile([128, 2 * 12 * 52], attn_dt, tag="pt", name="pt")
                if simsafe:
                    nc.vector.memset(pt[32:64, :], 0.0)
                    nc.vector.memset(pt[96:128, :], 0.0)
                exp_in = bass.AP(sp[:].tensor, sp[:].offset,
                                 [sp[:].ap[0], [512, 4], [N, 6], [1, N]])
                po_ = pt[:].rearrange("p f -> p f")
                exp_out = bass.AP(po_.tensor, po_.offset,
                                  [po_.ap[0], [52, 4], [4 * 52, 6], [1, N]])
                nc.scalar.activation(exp_out, exp_in, AF.Exp)
                # EB multiply (bias+mask in prob domain), alternating engine
                ptm = p_pt.tile([128, 2 * 12 * 52], attn_dt, tag="ptm", name="ptm")
                ea = et[:].rearrange("p f -> p f")
                eb_in = bass.AP(ea.tensor, ea.offset + ebase,
                                [ea.ap[0], [N, 4], [4 * N, 6], [1, N]])
                pt_in = bass.AP(po_.tensor, po_.offset,
                                [po_.ap[0], [52, 4], [4 * 52, 6], [1, N]])
                pm_ = ptm[:].rearrange("p f -> p f")
                for gg_ in range(2):
                    eb_in_g = bass.AP(ea.tensor, ea.offset + ebase + 12 * N * gg_,
                                      [ea.ap[0], [N, 4], [4 * N, 3], [1, N]])
                    pt_in_g = bass.AP(po_.tensor, po_.offset + 12 * 52 * gg_,
                                      [po_.ap[0], [52, 4], [4 * 52, 3], [1, N]])
                    ptm_out_g = bass.AP(pm_.tensor, pm_.offset + 12 * 52 * gg_,
                                        [pm_.ap[0], [52, 4], [4 * 52, 3], [1, N]])
                    nc.vector.tensor_tensor(ptm_out_g, pt_in_g, eb_in_g, ALU.mult)
                for gg in range(2):
                    g = 2 * gp + gg
                    vt = vts[g]
                    pbase = gg * 12 * 52
                    # AV (+ fused row-sums via ones column of vt)
                    av = ps_proj.tile([128, 12 * 33], f32, tag="psproj", name="psav", padded_shape=[128, 512])
                    if simsafe:
                        nc.vector.memset(av[32:64, :], 0.0)
                        nc.vector.memset(av[96:128, :], 0.0)
                    for s in range(2):
                        for h in range(NH):
                            nc.tensor.matmul(
                                av[64 * s:64 * s + 49, 33 * h:33 * h + 33],
                                ptm[64 * s:64 * s + 49,
                                    pbase + 52 * h:pbase + 52 * h + N],
                                vt[64 * s:64 * s + 49,
                                   396 * s + 33 * h:396 * s + 33 * h + 33],
                                start=True, stop=True,
                                tile_position=(64 * s, 64 * s))
                    # normalize
                    rt = p_rt.tile([128, 12], f32, tag="rt", name="rt")
                    nc.vector.reciprocal(
                        rt[:].rearrange("p (h e) -> p h e", e=1),
                        av[:].rearrange("p (h e) -> p h e", e=33)[:, :, 32:33])
                    on = p_on.tile([128, C], attn_dt, tag="onat", name="onat")
                    rap = rt[:]
                    rbc = bass.AP(rap.tensor, rap.offset, [rap.ap[0], [1, 12], [0, 32]])
                    nc.vector.tensor_tensor(
                        on[:].rearrange("p (h e) -> p h e", e=32),
                        av[:].rearrange("p (h e) -> p h e", e=33)[:, :, 0:32],
                        rbc, ALU.mult)
                    # transpose out [n, c] -> [c, n]: full-width, both windows at once
                    tp = ps_v.tile([128, 3 * 128], attn_dt, tag="psv", name="psv",
                                   padded_shape=[128, 512])
                    for cc in range(3):
                        nc.tensor.transpose(
                            tp[:, 128 * cc:128 * cc + 128],
                            on[:, cc * 128:(cc + 1) * 128],
                            idn[:])
                    oT = outT[:].rearrange("p (c t) -> p c t", c=3)
                    for s in range(2):
                        out_ap = bass.AP(oT.tensor, oT.offset + 98 * g + N * s,
                                         [oT.ap[0], [BAND, 3], [1, N]])
                        in_ap = bass.AP(tp[:].tensor, tp[:].offset + 64 * s,
                                        [tp[:].ap[0], [128, 3], [1, N]])
                        evac_copy(s % 2, out_ap, in_ap)
            # ---- output projection (+ window-major -> raster relayout)
            if d == 0:
                xm = [p_xm.tile([128, BAND], bf16, tag="xm", name="xm") for _ in range(3)]
                xm_tiles[bi] = xm
                ydst_t = xm
            else:
                yr = [p_yr.tile([128, BAND], f32, tag="yr", name="yr") for _ in range(3)]
                ydst_t = yr
            for half in range(2):
                for oc in range(3):
                    ps = ps_proj.tile([128, HALF], f32, tag="psproj", name="psproj", padded_shape=[128, 512])
                    for cc in range(3):
                        nc.tensor.matmul(
                            ps[:], wp[d][cc][:, oc * 128:(oc + 1) * 128],
                            outT[:, cc * BAND + half * HALF:
                                 cc * BAND + (half + 1) * HALF],
                            start=(cc == 0), stop=(cc == 2))
                    in_ap = ps[:].rearrange("p (w r j) -> p w r j",
                                            w=NG, r=WS, j=WS)
                    out_ap = ydst_t[oc][:].rearrange(
                        "p (r w j) -> p w r j", r=WS, w=NWB,
                        j=WS)[:, NG * half:NG * half + NG, :, :]
                    evac_bias((oc + half) % 2, out_ap, in_ap,
                              bp[d][:, oc:oc + 1])
            if d == 0:
                if bi == 0:
                    for cc in range(3):
                        nc.gpsimd.tensor_copy(stash[cc][:], xm_tiles[0][cc][:, 0:SHIFT * W])
            else:
                rows, cols = _band_ranges(bi, SHIFT)
                for oc in range(3):
                    yr3 = ydst_t[oc][:].rearrange("p (r c) -> p r c", r=WS)
                    dst3 = xout[oc * 128:(oc + 1) * 128, :].rearrange(
                        "p (r c) -> p r c", r=H)
                    for (dr, sr, nr) in rows:
                        for (dc, sc, ncl) in cols:
                            nc.sync.dma_start(dst3[:, sr:sr + nr, sc:sc + ncl],
                                              yr3[:, dr:dr + nr, dc:dc + ncl])

        for step in range(nbands + 2):
            if step < nbands:
                process_band(0, step)
            if step >= 2 and ndepth > 1:
                process_band(1, step - 2)

        for p in (ps_s, ps_v, ps_proj, p_stash, p_xm, p_rt, p_on, p_vt,
                  p_pt, p_yr, p_outT, p_qkb, p_xw1, p_xw0, p_xr, cpool):
            p.release()

    nc.compile()
    return nc


_NC = None


def _get_nc():
    global _NC
    if _NC is None:
        _NC = _build()
    return _NC


def _host_prep(qkv_w, qkv_b, proj_w, proj_b, rpb_table):
    scale = HD ** -0.5
    rpi = _rel_pos_index()
    masks = _masks()
    common = {}
    wqk = np.zeros((DEPTH, C, 2 * C), np.float32)
    wvv = np.zeros((DEPTH, C, C), np.float32)
    wpp = np.zeros((DEPTH, C, C), np.float32)
    bqk = np.zeros((DEPTH, 2 * C), np.float32)
    bpp = np.zeros((DEPTH, C), np.float32)
    for d in range(DEPTH):
        wq = qkv_w[d][:2 * C].T.copy()        # [C, 2C] (q then k)
        wq[:, :C] *= scale
        wqk[d] = wq
        wvv[d] = qkv_w[d][2 * C:].T
        wpp[d] = proj_w[d].T
        bq = qkv_b[d][:2 * C].copy()
        bq[:C] *= scale
        bqk[d] = bq
        bv = qkv_b[d][2 * C:]
        bpp[d] = proj_b[d] + proj_w[d] @ bv
    common["wqk0"] = _round_f32r(wqk[0])
    common["wqk1"] = wqk[1].astype(ml_dtypes.bfloat16)
    common["wv0"] = _round_f32r(wvv[0])
    common["wv1"] = wvv[1].astype(ml_dtypes.bfloat16)
    common["wp"] = wpp.astype(ml_dtypes.bfloat16)
    common["bqk"] = bqk
    common["bp"] = bpp

    # E tiles: rows 0-48 -> m, rows 64-112 -> m-64; value exp(bias[h,n,m]+mask[n,m])
    def etile(d, type_a, type_b):
        bias = rpb_table[d][rpi]              # [N, N, NH]
        t = np.zeros((128, 12 * N), np.float32)
        for s, ty in ((0, type_a), (1, type_b)):
            bm = bias + (masks[ty][:, :, None] if ty is not None else 0.0)
            ev = bm.transpose(2, 1, 0)   # [NH, m, n] (log domain)
            blk = ev.transpose(1, 0, 2).reshape(N, 12 * N)  # row m, col h*N+n
            t[64 * s:64 * s + N, :] = blk
        return t

    eb0 = np.exp(etile(0, None, None))
    common["e0"] = np.concatenate([eb0, eb0], axis=1).astype(ml_dtypes.bfloat16)
    # pair variants: [a|a], [a|c], [b|b], [b|d] where a,b,c,d = blocks
    # (0,0),(2,2),(0,1),(2,3) of the old layout
    ba = np.exp(etile(1, 0, 0))
    bb = np.exp(etile(1, 2, 2))
    bc = np.exp(etile(1, 0, 1))
    bd = np.exp(etile(1, 2, 3))
    e1 = np.concatenate([ba, ba, ba, bc, bb, bb, bb, bd], axis=1)
    common["e1"] = e1.astype(ml_dtypes.bfloat16)

    idn = np.zeros((128, 128), np.float32)
    for p in list(range(N)) + list(range(64, 64 + N)):
        idn[p, p] = 1.0
    common["idn"] = idn.astype(ml_dtypes.bfloat16)
    return common


def kernel(x, qkv_w, qkv_b, proj_w, proj_b, rpb_table, H=None, W=None):
    x = np.asarray(x, np.float32)
    qkv_w = np.asarray(qkv_w, np.float32)
    qkv_b = np.asarray(qkv_b, np.float32)
    proj_w = np.asarray(proj_w, np.float32)
    proj_b = np.asarray(proj_b, np.float32)
    rpb_table = np.asarray(rpb_table, np.float32)

    nc = _get_nc()
    common = _host_prep(qkv_w, qkv_b, proj_w, proj_b, rpb_table)
    in_maps = []
    for b in range(B):
        m = dict(common)
        m["xin"] = _round_f32r(np.ascontiguousarray(x[b].T))
        in_maps.append(m)
    res = run_bass_kernel_spmd(nc, in_maps, core_ids=list(range(B)))
    out = np.stack([np.ascontiguousarray(res.results[b]["xout"].T)
                    for b in range(B)])
    return out.astype(np.float32)
